# revision 36
# baseline (speedup 1.0000x reference)
"""Dilated attention (LongNet-style) Trainium2 kernel — v5 (DVE exp offload).

Problem: query/key/value (2, 8192, 12, 64) f32. Three dilation groups
(segment lengths 2048/4096/8192, dilation 1/2/4, head slices 0:4/4:8/8:12).
Each group's gather produces independent dense attention over 2048-position
dilated segments; outputs are normalized per (batch, head, channel) by the
sum over all segment positions, and divided by num_groups.

Sharding: 8 cores = 2 batches x 4 "head columns". Core c owns batch c//4 and
heads {j, 4+j, 8+j} where j = c%4 -- exactly 7 dense 2048x2048x64 attention
units per core (4 + 2 + 1 segments), perfectly balanced, with all segments of
any (batch, head) on one core so normalization needs no cross-core traffic.

v4 recap: K is an fp16 hi/lo pair packed along the contraction dim (scores =
(kh+kl)^T qh in ONE fp16 matmul — K errors are q-correlated and amplified
~150x by the final sum normalization, so the lo term is mandatory); Q and P
single fp16; V fp16 hi/lo packed as PV-matmul output rows [vh|vl|ones] so one
matmul yields numerator hi/lo and the softmax denominator.

v5: the v4 pipeline is ACT(exp)-bound wall-to-wall (1431ns per 3-unit round,
215.8us stream) while the PE needs only 191.2us. Changes:
  1. ~20% of exp rounds run on the *vector* engine via two custom DVE ops:
     pass1 p = (1+z) + z^2(c2 + c3 z + c4 z^2)  (z = score, pre-scaled so
     z = arg/16 lands directly out of the matmul: Q is host-scaled by 1/128,
     K unscaled), pass2 p^16 * 64 by four squarings. Max rel err vs exp:
     2.5e-4 + fp16 out quantization — below the fp16-P noise already present.
     End-to-end sim: 5.8e-3 (thr 2e-2).
  2. Rounds shrink to 2 units (RW=2) so PSUM fits THREE rotating score
     buffers (3x2 banks + 2 PV banks = 8): a DVE round's slower pass1 then
     never stalls the ACT stream's buffer rotation.
  3. PV accumulators DMA straight from PSUM to DRAM per 512-col chunk
     (no SBUF staging, no DVE copies; the bank has an ~8-round idle window).
  4. Startup: the ACT exp-table load fires at t~0 from a tiny SBUF dummy
     (before, it hid behind a 12us DMA gate); the first segment's DMAs are
     split and ordered k0|q0|v0 piecewise so round 0 starts ~4us earlier.
With ACT+DVE jointly covering exp, the Tensor engine's 896 x 512-col fp16
matmuls (854ns/round) become the critical path.

Host: num = O'[0:64] (+= O'[64:127] for ch<63), den = O'[127], T = num/den,
then the group normalization (sum over positions) and /3, scattered into
the (2, 8192, 12, 64) output. Positions not in a dilated group stay zero.
"""

import math
import os
import sys

if "/opt/trn_rl_repo" not in sys.path:
    sys.path.insert(0, "/opt/trn_rl_repo")
if "jax" not in sys.modules:
    os.environ.setdefault("JAX_PLATFORMS", "axon")

import numpy as np

import concourse.bass as bass  # noqa: F401
import concourse.mybir as mybir
import concourse.tile as tile
from concourse import bacc
from concourse.bass_utils import run_bass_kernel_spmd

F32 = mybir.dt.float32
F16 = mybir.dt.float16

B, N, H, D = 2, 8192, 12, 64
NSEG = 7           # segments per core
SEG = 2048         # dilated segment length
NCHUNK = NSEG * 4  # 512-wide q chunks per core
NKB = 16           # 128-row k blocks per segment
NUNIT = NCHUNK * NKB
RW = 2             # units per round (score tile = 2 PSUM banks, 3 buffers)
NR = NUNIT // RW
QSC_Q = float(1.0 / 256.0)  # q pre-scale: scores come out as z = arg/32
VSC = np.float32(256.0)     # v pre-scale (cancels in num/den)
PBIAS = float(math.log(64.0))  # ACT path: exp(32 z + ln64) = 64 e^arg

# DVE poly-exp: e^z ~ (1+z) + z^2(c2 + c3 z + c4 z^2) on |z| <= 0.21,
# then ^32 (5 squarings) * 64.  Rel err 4.4e-7, ^32 -> 1.6e-5 —
# negligible next to the fp16 quantization of P.
PC2 = 0.5000069832135483
PC3 = 0.16698561866339273
PC4 = 0.04152138113462383


def _dve_round(r):
    # split rounds: DVE handles the round's first 512-block, ACT the second
    return r % 2 == 0 and 2 <= r < NR - 4


# ---- custom DVE op registration (runtime, self-contained) ----------------

def _register_dve_ops():
    from concourse import dve_ops
    from concourse.dve_spec import (
        Spec, Src0, C0, C1, C2, One, sq, lower, _has_src1,
    )
    from concourse.dve_uop import DveOpSpec

    def reg(name, body, reference):
        if name in dve_ops._SUB_OPCODE_FOR_NAME:
            return next(o for o in dve_ops.OPS if o.name == name)
        row = max(dve_ops._SUB_OPCODE_FOR_NAME.values()) + 1
        assert row < 0x20, "custom-DVE opcode rows exhausted"
        dve_ops._SUB_OPCODE_FOR_NAME[name] = row
        spec = Spec(body=body, reference=reference)
        shas = {}
        for ver in ("v3", "v4"):
            uops = lower(spec, ver=ver)
            shas[ver] = DveOpSpec(
                name=name, opcode=row, uops=uops, rd1_en=_has_src1(spec)
            ).sha(ver)
        op = dve_ops.DveOp(name, spec, subdim=False, uops_sha=shas)
        dve_ops.OPS.append(op)
        dve_ops.CUSTOM_DVE_SPECS[name] = spec
        return op

    # p = (1+z) + z^2((c3 z + c2) + z^2 c4);  C0=c3 C1=c2 C2=c4 (8 ALU ops)
    f2 = sq(Src0)
    body1 = (One + Src0) + f2 * ((Src0 * C0 + C1) + f2 * C2)

    def ref1(in0, in1, s0, s1, imm2):
        z = in0.astype(np.float32)
        return ((1 + z) + z * z * ((z * np.float32(s0) + np.float32(s1))
                                  + z * z * np.float32(imm2))).astype(np.float32)

    body2 = sq(sq(sq(sq(sq(Src0))))) * C0

    def ref2(in0, in1, s0, s1, imm2):
        p = in0.astype(np.float32)
        for _ in range(5):
            p = p * p
        return p * np.float32(s0)

    return reg("EXP32_POLY1_ANT", body1, ref1), reg("EXP32_POLY2_ANT", body2, ref2)


_CACHE = {}
LAST_RESULT = {}


def _build_nc():
    exp_p1, exp_p2 = _register_dve_ops()

    nc = bacc.Bacc("TRN2", target_bir_lowering=False, debug=False,
                   enable_asserts=False, num_devices=8)
    qhh = nc.dram_tensor("qhh", [128, NSEG * SEG], F16, kind="ExternalInput")
    khl = nc.dram_tensor("khl", [128, NSEG * SEG], F16, kind="ExternalInput")
    vhl = nc.dram_tensor("vhl", [128, NSEG * NKB * 128], F16,
                         kind="ExternalInput")
    out = nc.dram_tensor("out", [128, NCHUNK * 512], F32, kind="ExternalOutput")
    qhh_ap, khl_ap, vhl_ap, out_ap = qhh.ap(), khl.ap(), vhl.ap(), out.ap()

    with tile.TileContext(nc) as tc:
        with (
            tc.tile_pool(name="inp", bufs=1) as inp,
            tc.tile_pool(name="pt", bufs=5) as ptp,
            tc.tile_pool(name="mid", bufs=2) as midp,
            tc.tile_pool(name="osb", bufs=3) as osbp,
            tc.tile_pool(name="score", bufs=3, space="PSUM") as scp,
            tc.tile_pool(name="ot", bufs=2, space="PSUM") as otp,
        ):
            bias_t = inp.tile([128, 1], F32, tag="bias", name="bias_t")
            nc.vector.memset(bias_t[:, :], PBIAS)

            # Fire the ~2.7us ACT exp-table load immediately: the dummy's
            # input is zeroed by the Scalar engine itself (memzero), so the
            # chain never waits on another engine's prologue.
            wsm = inp.tile([128, 16], F32, tag="wsm", name="wsm")
            nc.scalar.memzero(wsm[:, :])
            wp0 = inp.tile([128, 16], F16, tag="wp0", name="wp0")
            nc.scalar.activation(
                wp0[:, :], wsm[:, :],
                mybir.ActivationFunctionType.Exp, scale=32.0)

            # PE clock-ramp starter: the DVFS ramp is time-based from the
            # first matmul, so a burst of *small* (128-col) dummies starts
            # the clock without blocking round 0 in the PE queue.
            wsrc = inp.tile([128, 128], F16, tag="wsrc", name="wsrc")
            wjunk = inp.tile([128, 512], F16, tag="wjunk", name="wjunk")
            nc.vector.memset(wsrc[:, :], 0.01)
            nc.vector.memset(wjunk[:, :], 0.01)
            warm = otp.tile([128, 512], F32, tag="ot", name="warm")
            for i in range(9):
                nc.tensor.matmul(warm[:, :128], wsrc[:, :], wjunk[:, :128],
                                 start=(i == 0), stop=(i == 8))

            qh_sb, k_sb, v_sb = [], [], []
            for s in range(NSEG):
                qh = inp.tile([128, SEG], F16, tag=f"qh{s}", name=f"qh{s}")
                kk = inp.tile([128, SEG], F16, tag=f"k{s}", name=f"k{s}")
                vv = inp.tile([128, NKB * 128], F16, tag=f"v{s}", name=f"v{s}")
                qh_sb.append(qh)
                k_sb.append(kk)
                v_sb.append(vv)
            # Input DMAs: K/V issue from the otherwise-idle GpSimd engine,
            # Q from Sync — two issue queues, so round 0's q-chunk and
            # k-block transfer concurrently right after the ~8us prologue.
            def piece(eng, t, ap_, z):
                eng.dma_start(t[:, z * 512:(z + 1) * 512],
                              ap_[:, z * 512:(z + 1) * 512])
            for z in range(4):
                piece(nc.sync, qh_sb[0], qhh_ap, z)
            for z in range(4):
                piece(nc.gpsimd, k_sb[0], khl_ap, z)
                piece(nc.gpsimd, v_sb[0], vhl_ap, z)
            for s in range(1, NSEG):
                nc.sync.dma_start(qh_sb[s][:, :],
                                  qhh_ap[:, s * SEG:(s + 1) * SEG])
                for t, ap_ in ((k_sb[s], khl_ap), (v_sb[s], vhl_ap)):
                    nc.gpsimd.dma_start(t[:, :], ap_[:, s * SEG:(s + 1) * SEG])

            ot_tiles = {}
            pend = [[], [], []]  # PV work lagged by 1..3 rounds
            pend_out = []        # chunks whose PSUM->SBUF copy is deferred

            def emit_copies():
                # DMA cannot source PSUM; stage chunks in SBUF via DVE
                # copies and ship each immediately. Deferred to just after a
                # DVE round's pass2 so a not-yet-ready copy never head-of-
                # line-blocks the next pass1 in the in-order DVE queue.
                while pend_out:
                    cid = pend_out.pop(0)
                    oc = osbp.tile([128, 512], F32, tag="oc", name=f"oc{cid}")
                    nc.vector.tensor_copy(oc[:, :], ot_tiles[cid][:, :])
                    nc.sync.dma_start(
                        out_ap[:, cid * 512:(cid + 1) * 512], oc[:, :])

            def flush(items):
                for p1ref, i, u in items:
                    cid, kb = divmod(u, NKB)
                    s, _c = divmod(cid, 4)
                    if kb == 0:
                        ot_tiles[cid] = otp.tile([128, 512], F32, tag="ot",
                                                 name=f"ot{cid}")
                    vsl = slice(kb * 128, (kb + 1) * 128)
                    psl = slice(i * 512, (i + 1) * 512)
                    nc.tensor.matmul(ot_tiles[cid][:, :], v_sb[s][:, vsl],
                                     p1ref[:, psl],
                                     start=(kb == 0), stop=(kb == NKB - 1))
                    if kb == NKB - 1:
                        pend_out.append(cid)

            for r in range(NR):
                units = range(r * RW, (r + 1) * RW)
                score = scp.tile([128, 512 * RW], F32, tag="score",
                                 name=f"score{r}")
                for i, u in enumerate(units):
                    cid, kb = divmod(u, NKB)
                    s, c = divmod(cid, 4)
                    osl = slice(i * 512, (i + 1) * 512)
                    nc.tensor.matmul(score[:, osl],
                                     k_sb[s][:, kb * 128:(kb + 1) * 128],
                                     qh_sb[s][:, c * 512:(c + 1) * 512],
                                     start=True, stop=True)
                p1 = ptp.tile([128, 512 * RW], F16, tag="p1", name=f"p1_{r}")
                if _dve_round(r):
                    # split round: DVE exps cols 0-511 while ACT exps cols
                    # 512-1023 of the same tile — the PSUM banks free after
                    # ~760ns, so the 3-buffer rotation never stalls.
                    mid = midp.tile([128, 512], F32, tag="mid",
                                    name=f"mid{r}")
                    nc.vector._custom_dve(exp_p1, out=mid[:, :],
                                          in0=score[:, :512],
                                          s0=PC3, s1=PC2, imm2=PC4)
                    nc.scalar.activation(
                        p1[:, 512:], score[:, 512:],
                        mybir.ActivationFunctionType.Exp, scale=32.0,
                        bias=bias_t[:, :])
                    nc.vector._custom_dve(exp_p2, out=p1[:, :512],
                                          in0=mid[:, :], s0=64.0)
                    emit_copies()
                elif r >= NR - 6:
                    emit_copies()
                    nc.scalar.activation(
                        p1[:, :], score[:, :],
                        mybir.ActivationFunctionType.Exp, scale=32.0,
                        bias=bias_t[:, :])
                else:
                    nc.scalar.activation(
                        p1[:, :], score[:, :],
                        mybir.ActivationFunctionType.Exp, scale=32.0,
                        bias=bias_t[:, :])
                flush(pend[2])
                pend = [[(p1, i, u) for i, u in enumerate(units)],
                        pend[0], pend[1]]
            for items in reversed(pend):
                flush(items)
            emit_copies()

    nc.compile()
    return nc


def _prep_core(query, key, value, core):
    b, j = divmod(core, 4)
    segs = []
    for arr in (query, key, value):
        h0 = arr[b, :, j, :].reshape(4, SEG, D)
        h1 = arr[b, :, 4 + j, :].reshape(2, 4096, D)[:, 1::2, :]
        h2 = arr[b, 2::4, 8 + j, :][None]
        segs.append(np.concatenate([h0, h1, h2], axis=0))  # [7, 2048, 64]
    qs, ks, vs = segs
    # [64, NSEG*SEG] with col = s*SEG + p
    qt = (qs * QSC_Q).transpose(2, 0, 1).reshape(D, NSEG * SEG)
    kt = ks.transpose(2, 0, 1).reshape(D, NSEG * SEG)
    qh = qt.astype(np.float16)
    kh = kt.astype(np.float16)
    kl = (kt - kh).astype(np.float16)
    vv = vs * VSC  # [7, 2048, 64] f32, pre-scaled
    v1h = vv.astype(np.float16)
    v1l = (vv - v1h).astype(np.float16)
    # packed stationary blocks: [vh(64) | vl(ch 0-62) | ones] per k-block
    blk = np.empty((NSEG, SEG, 128), np.float16)
    blk[:, :, 0:64] = v1h
    blk[:, :, 64:127] = v1l[:, :, 0:63]
    blk[:, :, 127] = np.float16(256.0)
    vhl = blk.reshape(NSEG, NKB, 128, 128).transpose(2, 0, 1, 3).reshape(128, -1)
    return {
        "qhh": np.ascontiguousarray(np.concatenate([qh, qh], axis=0)),
        "khl": np.ascontiguousarray(np.concatenate([kh, kl], axis=0)),
        "vhl": np.ascontiguousarray(vhl),
    }


def _unshard(results, dtype):
    full = np.zeros((B, N, H, D), dtype)
    for core in range(8):
        b, j = divmod(core, 4)
        o = results[core]["out"].astype(np.float64)
        num = o[0:64].copy()
        num[0:63] += o[64:127]
        T = num / o[127:128]  # [64, 14336]
        h0 = T[:, :4 * SEG]
        full[b, :, j, :] = (h0 / (3.0 * h0.sum(1, keepdims=True))).T
        h1 = T[:, 4 * SEG:6 * SEG]
        h1 = h1 / (3.0 * h1.sum(1, keepdims=True))
        for g in range(2):
            full[b, g * 4096 + 1:(g + 1) * 4096:2, 4 + j, :] = \
                h1[:, g * SEG:(g + 1) * SEG].T
        h2 = T[:, 6 * SEG:]
        full[b, 2::4, 8 + j, :] = (h2 / (3.0 * h2.sum(1, keepdims=True))).T
    return full


def _ensure_axon_backend():
    """The bass PJRT path needs the axon/neuron jax backend. A harness may
    pin JAX_PLATFORMS=cpu for its reference; re-select axon if so."""
    import jax
    try:
        plat = jax.devices()[0].platform
    except Exception:
        plat = ""
    if plat not in ("axon", "neuron"):
        try:
            jax.config.update("jax_platforms", "axon,cpu")
            jax.devices()
        except Exception:
            pass


def kernel(query, key, value):
    _ensure_axon_backend()
    query = np.asarray(query, np.float32)
    key = np.asarray(key, np.float32)
    value = np.asarray(value, np.float32)
    assert query.shape == (B, N, H, D)

    if "nc" not in _CACHE:
        _CACHE["nc"] = _build_nc()
    nc = _CACHE["nc"]

    in_maps = [_prep_core(query, key, value, c) for c in range(8)]
    res = run_bass_kernel_spmd(nc, in_maps, core_ids=list(range(8)))
    LAST_RESULT["exec_time_ns"] = res.exec_time_ns
    return _unshard(res.results, query.dtype)
